# revision 2
# baseline (speedup 1.0000x reference)
"""Bass/Trainium2 kernel for nn_DecoderBlock (masked block-sparse linear +
BatchNorm(train) + Swish), sharded over C_OUT blocks across 8 NeuronCores.

Contract: kernel(**inputs) takes the FULL inputs from setup_inputs() and
returns the FULL [B, C_OUT, F_OUT] output.

Sharding: core k owns output channels [4k, 4k+4). With the reference's
block mask (o//4 == c//4) each core needs only input channels [4k, 4k+4),
so the useful slice of W (1/8 of it) is read from HBM exactly once across
the 8 cores, and every core holds the whole batch for its features =>
BatchNorm statistics are local (no collectives).

Math notes:
 - bias cancels exactly through BatchNorm's mean subtraction -> dropped.
 - matmul runs as 3 bf16 passes (W_hi@x_hi + W_hi@x_lo + W_lo@x_hi) which
   is ~2^-17 relative accuracy, faster than native fp32 (4 cycles/row).
 - epilogue: y -> bn_stats/bn_aggr (mean/var over batch on the free dim),
   then one ScalarE Silu(a*y + c) with per-partition a = gamma*rstd,
   c = beta - mean*a.
"""

import numpy as np
import ml_dtypes

B = 256
C_IN, F_IN = 32, 256
C_OUT, F_OUT = 32, 256
KERNEL_SIZE = 4
BN_EPS = 1e-5
N_CORES = 8
OC_PER_CORE = C_OUT // N_CORES  # 4 output channels per core
P = 128

TRACE = False  # set True (e.g. from test.py) to capture an NTFF profile
LAST_RESULT = {}  # exec_time_ns etc. from the most recent run

_program_cache = {}


def _build_program(kc):
    """Build the SPMD Bass program for kc active input channels per core."""
    import concourse.bass as bass
    import concourse.tile as tile
    import concourse.mybir as mybir

    K = kc * F_IN  # contraction dim
    KT = K // P  # k-tiles of 128
    PT = (OC_PER_CORE * F_OUT) // P  # output-feature tiles of 128 (=8)
    NP = OC_PER_CORE * F_OUT  # per-core output features (=1024)
    f32 = mybir.dt.float32
    bf16 = mybir.dt.bfloat16

    nc = bass.Bass()
    xh_d = nc.declare_dram_parameter("xh", [KT, P, B], bf16, isOutput=False)
    xl_d = nc.declare_dram_parameter("xl", [KT, P, B], bf16, isOutput=False)
    wh_d = nc.declare_dram_parameter("wh", [KT, P, NP], bf16, isOutput=False)
    wl_d = nc.declare_dram_parameter("wl", [KT, P, NP], bf16, isOutput=False)
    gs_d = nc.declare_dram_parameter("gs", [P, PT], f32, isOutput=False)
    bs_d = nc.declare_dram_parameter("bs", [P, PT], f32, isOutput=False)
    out_d = nc.declare_dram_parameter("out", [PT, P, B], f32, isOutput=True)

    with tile.TileContext(nc) as tc:
        with (
            tc.tile_pool(name="wpool", bufs=1) as wpool,
            tc.tile_pool(name="xpool", bufs=1) as xpool,
            tc.tile_pool(name="spool", bufs=1) as spool,
            tc.tile_pool(name="stat", bufs=1) as stat,
            tc.tile_pool(name="opool", bufs=1) as opool,
            tc.tile_pool(name="psum", bufs=1, space="PSUM") as psum,
        ):
            eps_t = spool.tile([P, 1], f32, name="eps")
            nc.vector.memset(eps_t, BN_EPS)
            gs_t = spool.tile([P, PT], f32, name="gs")
            nc.sync.dma_start(out=gs_t, in_=gs_d.ap())
            bs_t = spool.tile([P, PT], f32, name="bs")
            nc.sync.dma_start(out=bs_t, in_=bs_d.ap())

            xh_t, xl_t, wh_t, wl_t = [], [], [], []
            for kt in range(KT):
                t = xpool.tile([P, B], bf16, name=f"xh{kt}")
                nc.sync.dma_start(out=t, in_=xh_d.ap()[kt])
                xh_t.append(t)
                t = xpool.tile([P, B], bf16, name=f"xl{kt}")
                nc.sync.dma_start(out=t, in_=xl_d.ap()[kt])
                xl_t.append(t)
                t = wpool.tile([P, NP], bf16, name=f"wh{kt}")
                nc.sync.dma_start(out=t, in_=wh_d.ap()[kt])
                wh_t.append(t)
                t = wpool.tile([P, NP], bf16, name=f"wl{kt}")
                nc.sync.dma_start(out=t, in_=wl_d.ap()[kt])
                wl_t.append(t)

            ps = [psum.tile([P, B], f32, name=f"ps{pt}") for pt in range(PT)]

            for kt in range(KT):
                for pt in range(PT):
                    whs = wh_t[kt][:, pt * P : (pt + 1) * P]
                    wls = wl_t[kt][:, pt * P : (pt + 1) * P]
                    first = kt == 0
                    last = kt == KT - 1
                    nc.tensor.matmul(
                        ps[pt], lhsT=whs, rhs=xh_t[kt], start=first, stop=False
                    )
                    nc.tensor.matmul(
                        ps[pt], lhsT=whs, rhs=xl_t[kt], start=False, stop=False
                    )
                    nc.tensor.matmul(
                        ps[pt], lhsT=wls, rhs=xh_t[kt], start=False, stop=last
                    )

            for pt in range(PT):
                stats = stat.tile([P, 6], f32, name=f"stats{pt}")
                nc.vector.bn_stats(out=stats, in_=ps[pt])
                mv = stat.tile([P, 2], f32, name=f"mv{pt}")
                nc.vector.bn_aggr(out=mv, in_=stats)
                std = stat.tile([P, 1], f32, name=f"std{pt}")
                nc.scalar.activation(
                    out=std,
                    in_=mv[:, 1:2],
                    func=mybir.ActivationFunctionType.Sqrt,
                    bias=eps_t,
                    scale=1.0,
                )
                rstd = stat.tile([P, 1], f32, name=f"rstd{pt}")
                nc.vector.reciprocal(out=rstd, in_=std)
                a_t = stat.tile([P, 1], f32, name=f"a{pt}")
                nc.vector.tensor_mul(out=a_t, in0=gs_t[:, pt : pt + 1], in1=rstd)
                ma_t = stat.tile([P, 1], f32, name=f"ma{pt}")
                nc.vector.tensor_mul(out=ma_t, in0=mv[:, 0:1], in1=a_t)
                c_t = stat.tile([P, 1], f32, name=f"c{pt}")
                nc.vector.tensor_sub(out=c_t, in0=bs_t[:, pt : pt + 1], in1=ma_t)
                o_t = opool.tile([P, B], f32, name=f"o{pt}")
                nc.scalar.activation(
                    out=o_t,
                    in_=ps[pt],
                    func=mybir.ActivationFunctionType.Silu,
                    bias=c_t,
                    scale=a_t,
                )
                nc.sync.dma_start(out=out_d.ap()[pt], in_=o_t)

    _split_excess_waits(nc)
    return nc


def _split_excess_waits(nc, limit=1):
    """Walrus codegen rejects instructions carrying more than one sync wait;
    hoist excess waits onto same-engine NOPs inserted immediately before."""
    import concourse.mybir as mybir

    for fn in nc.m.functions:
        for blk in fn.blocks:
            new_insts = []
            for inst in blk.instructions:
                si = inst.sync_info
                waits = list(si.on_wait) if (si and si.on_wait) else []
                if len(waits) > limit:
                    extra = waits[:-limit]
                    inst.sync_info.on_wait = waits[-limit:]
                    while extra:
                        chunk, extra = extra[:limit], extra[limit:]
                        nop = mybir.InstNoOp(
                            name=nc.get_next_instruction_name(),
                            engine=inst.engine,
                            ins=[],
                            outs=[],
                            sync_info=mybir.SyncInfo(on_wait=chunk, on_update=[]),
                        )
                        new_insts.append(nop)
                new_insts.append(inst)
            blk.instructions[:] = new_insts


def _hi_lo(a):
    hi = a.astype(ml_dtypes.bfloat16)
    lo = (a - hi.astype(np.float32)).astype(ml_dtypes.bfloat16)
    return hi, lo


def kernel(x, W, bias, gamma, beta, mask):
    from concourse.bass_utils import run_bass_kernel_spmd

    x = np.asarray(x, dtype=np.float32)
    W = np.asarray(W, dtype=np.float32)
    gamma = np.asarray(gamma, dtype=np.float32)
    beta = np.asarray(beta, dtype=np.float32)
    mask_np = np.asarray(mask).astype(bool)

    groups = [
        list(range(OC_PER_CORE * k, OC_PER_CORE * (k + 1))) for k in range(N_CORES)
    ]
    active = [np.where(mask_np[g].any(axis=0))[0] for g in groups]
    kc = max(1, max(len(a) for a in active))

    key = kc
    if key not in _program_cache:
        _program_cache[key] = _build_program(kc)
    nc = _program_cache[key]

    K = kc * F_IN
    KT = K // P
    PT = (OC_PER_CORE * F_OUT) // P
    NP = OC_PER_CORE * F_OUT

    gamma2 = gamma.reshape(C_OUT, F_OUT)
    beta2 = beta.reshape(C_OUT, F_OUT)

    in_maps = []
    for k in range(N_CORES):
        g = groups[k]
        a = active[k]
        w_eff = np.zeros((OC_PER_CORE, kc, F_OUT, F_IN), dtype=np.float32)
        if len(a):
            w_eff[:, : len(a)] = (
                W[g][:, a] * mask_np[g][:, a][:, :, None, None]
            )
        # [k=(j,i), p=(o_local,f)]
        wT = np.ascontiguousarray(
            w_eff.transpose(1, 3, 0, 2).reshape(K, NP)
        )
        xb = np.zeros((B, kc, F_IN), dtype=np.float32)
        if len(a):
            xb[:, : len(a)] = x[:, a, :]
        xT = np.ascontiguousarray(xb.transpose(1, 2, 0).reshape(K, B))

        wh, wl = _hi_lo(wT)
        xh, xl = _hi_lo(xT)

        g_core = gamma2[g].reshape(NP)  # ordered (o_local, f) = p
        b_core = beta2[g].reshape(NP)
        gs = np.ascontiguousarray(g_core.reshape(PT, P).T)  # [P, PT]
        bs = np.ascontiguousarray(b_core.reshape(PT, P).T)

        in_maps.append(
            {
                "xh": np.ascontiguousarray(xh.reshape(KT, P, B)),
                "xl": np.ascontiguousarray(xl.reshape(KT, P, B)),
                "wh": np.ascontiguousarray(wh.reshape(KT, P, NP)),
                "wl": np.ascontiguousarray(wl.reshape(KT, P, NP)),
                "gs": gs,
                "bs": bs,
            }
        )

    res = run_bass_kernel_spmd(
        nc, in_maps, core_ids=list(range(N_CORES)), trace=TRACE
    )
    LAST_RESULT["exec_time_ns"] = res.exec_time_ns
    LAST_RESULT["mean_exec_time_ns"] = res.mean_exec_time_ns
    LAST_RESULT["trace"] = res.instructions_and_trace

    out = np.empty((B, C_OUT, F_OUT), dtype=np.float32)
    for k in range(N_CORES):
        y = res.results[k]["out"].reshape(NP, B)  # [p, b]
        out[:, groups[k], :] = y.T.reshape(B, OC_PER_CORE, F_OUT)
    return out
